# revision 1
# baseline (speedup 1.0000x reference)
"""Tensor-parallel GQA attention forward for one TRN2 chip (8 NeuronCores).

Strategy (8-way tensor parallel over heads):
  - each core owns 4 q-heads + 1 kv-head (wq/wk/wv column-sharded, host side)
  - x is transposed on-device: each core PE-transposes its 256-row slice of x
    (cast to bf16) and an AllGather assembles the full xT on every core
  - projections produce qT/kT (head_dim on partitions) and v (natural layout)
    directly in the layouts the attention matmuls want; RoPE is applied in a
    de-interleaved head-dim ordering (dot products are permutation invariant)
  - scores are computed transposed (S^T[k, q]) so exp runs straight out of
    PSUM; softmax denominators come for free as a 65th column of ones in the
    PV matmul; causal masking = skipping k-tiles above the diagonal plus a
    0/1 pattern multiply on the 4 diagonal-band tiles per chunk
  - an AllToAll flips head-sharded attnT to sequence-sharded, each core then
    computes its 256-row slice of the output projection against full wo
  - compute dtype bf16 (fp32 PSUM accumulation), output fp32
"""

import numpy as np

NC_CORES = 8
SEQ = 2048
DIM = 2048
HD = 64            # head dim
LHEADS = 4         # q heads per core
SC = SEQ // NC_CORES   # 256: sequence rows per core (transpose shard / output shard)
CH = 512           # q-chunk width for attention
NCH = SEQ // CH    # 4
KT = SEQ // 128    # 16 k-tiles
DT = DIM // 128    # 16 d-tiles

_CACHE = {}


def _build_nc():
    import concourse.bass as bass
    import concourse.mybir as mybir
    import concourse.tile as tile
    from concourse import bacc
    from concourse.masks import make_identity

    BF = mybir.dt.bfloat16
    F32 = mybir.dt.float32
    MUL = mybir.AluOpType.mult
    ADD = mybir.AluOpType.add
    SUB = mybir.AluOpType.subtract

    nc = bacc.Bacc("TRN2", target_bir_lowering=False, debug=False,
                   num_devices=NC_CORES)

    # ---- external I/O (per-core shards) ----
    # W_all columns: [q-pair0: 8x128 | q-pair1: 8x128 | k: 8x64 | v: 8x64]
    x_sl = nc.dram_tensor("x_sl", [SC, DIM], F32, kind="ExternalInput")
    w_all = nc.dram_tensor("w_all", [DIM, DIM + 2 * 512], BF, kind="ExternalInput")
    wo = nc.dram_tensor("wo", [DIM, DIM], BF, kind="ExternalInput")
    cosR = nc.dram_tensor("cosR", [SC, 32], F32, kind="ExternalInput")
    sinR = nc.dram_tensor("sinR", [SC, 32], F32, kind="ExternalInput")
    out = nc.dram_tensor("out", [SC, DIM], F32, kind="ExternalOutput")

    groups = [list(range(NC_CORES))]
    WCOLS = DIM + 1024          # 3072
    NCH_W = WCOLS // CH         # 6 projection column chunks

    with tile.TileContext(nc) as tc:
        # DRAM bounce buffers for collectives
        apkv_in, _ = tc.tile([NC_CORES, SC, 128], BF, space=bass.MemorySpace.DRAM,
                             name="apkv_in")
        apkv_out, _ = tc.tile([NC_CORES, SC, 128], BF, space=bass.MemorySpace.DRAM,
                              addr_space="Shared", name="apkv_out")
        apq0_in, _ = tc.tile([NC_CORES, SC, 128], BF, space=bass.MemorySpace.DRAM,
                             name="apq0_in")
        apq0_out, _ = tc.tile([NC_CORES, SC, 128], BF, space=bass.MemorySpace.DRAM,
                              addr_space="Shared", name="apq0_out")
        apq1_in, _ = tc.tile([NC_CORES, SC, 128], BF, space=bass.MemorySpace.DRAM,
                             name="apq1_in")
        apq1_out, _ = tc.tile([NC_CORES, SC, 128], BF, space=bass.MemorySpace.DRAM,
                              addr_space="Shared", name="apq1_out")
        a2a_in0, _ = tc.tile([NC_CORES, 128, SC], BF,
                             space=bass.MemorySpace.DRAM, name="a2a_in0")
        a2a_out0, _ = tc.tile([NC_CORES, 128, SC], BF,
                              space=bass.MemorySpace.DRAM,
                              addr_space="Shared", name="a2a_out0")
        a2a_in1, _ = tc.tile([NC_CORES, 128, SC], BF,
                             space=bass.MemorySpace.DRAM, name="a2a_in1")
        a2a_out1, _ = tc.tile([NC_CORES, 128, SC], BF,
                              space=bass.MemorySpace.DRAM,
                              addr_space="Shared", name="a2a_out1")

        with tc.tile_pool(name="persist", bufs=1) as pp, \
             tc.tile_pool(name="wstream", bufs=2) as wsp, \
             tc.tile_pool(name="work", bufs=2) as wp, \
             tc.tile_pool(name="psum", bufs=2, space="PSUM") as psp:

            # ---------------- local transpose of own x slice ----------------
            ident = pp.tile([128, 128], BF, name="ident")
            make_identity(nc, ident[:])

            xsl_bf = pp.tile([128, 2, DIM], BF, name="xsl_bf")
            for pt in range(2):
                nc.gpsimd.dma_start(
                    xsl_bf[:, pt, :], x_sl[128 * pt:128 * pt + 128, :])

            xTc = pp.tile([128, DT, SC], BF, name="xTc")
            for pt in range(2):
                for j in range(DT):
                    trp = psp.tile([128, 128], BF, tag="ps", bufs=4, name="trp")
                    nc.tensor.transpose(trp[:], xsl_bf[:, pt, 128 * j:128 * j + 128],
                                        ident[:])
                    nc.vector.tensor_copy(xTc[:, j, 128 * pt:128 * pt + 128], trp[:])

            # rope tables, replicated across the 40 roped heads (32 q + 8 k),
            # per local 128-row s-tile
            cosR_sb = pp.tile([128, 2, 32], BF, name="cosR_sb")
            sinR_sb = pp.tile([128, 2, 32], BF, name="sinR_sb")
            nc.gpsimd.dma_start(cosR_sb[:], cosR[:].rearrange("(t p) f -> p t f", p=128))
            nc.gpsimd.dma_start(sinR_sb[:], sinR[:].rearrange("(t p) f -> p t f", p=128))
            cos_rep = pp.tile([128, 2, 8, 32], BF, name="cos_rep")
            sin_rep = pp.tile([128, 2, 8, 32], BF, name="sin_rep")
            for st in range(2):
                for h in range(8):
                    nc.vector.tensor_copy(cos_rep[:, st, h, :], cosR_sb[:, st, :])
                    nc.vector.tensor_copy(sin_rep[:, st, h, :], sinR_sb[:, st, :])

            # ---------------- seq-sharded projections (all heads, own 256 s) ----
            # W chunk order: k, v first (their A2A overlaps the q projections),
            # then q-pair0, then q-pair1; each section's AllToAll is issued as
            # soon as its columns are projected + roped.
            proj = pp.tile([128, 2, WCOLS], BF, name="proj")

            def proj_chunk(ch):
                wt = wsp.tile([128, DT, CH], BF, tag="wt", bufs=2, name="wt")
                for hf in range(2):
                    nc.sync.dma_start(
                        wt[:, 8 * hf:8 * hf + 8, :],
                        w_all[1024 * hf:1024 * hf + 1024, CH * ch:CH * ch + CH]
                        .rearrange("(t p) m -> p t m", p=128))
                for st in range(2):
                    psq = psp.tile([128, CH], F32, tag="ps", bufs=4, name="psq")
                    for dt in range(DT):
                        nc.tensor.matmul(
                            psq[:], xTc[:, dt, 128 * st:128 * st + 128],
                            wt[:, dt, :],
                            start=(dt == 0), stop=(dt == DT - 1))
                    if ch < 5:   # q and k columns get RoPE (8 head-pairs/chunk)
                        nh = 8
                        pv = psq[:].rearrange("p (h x) -> p h x", x=32)
                        ta = wp.tile([128, 8, 32], F32, tag="ropeA", bufs=2, name="ta")
                        tb = wp.tile([128, 8, 32], F32, tag="ropeB", bufs=2, name="tb")
                        dstv = proj[:, st, CH * ch:CH * ch + CH].rearrange(
                            "p (h x) -> p h x", x=32)
                        crep = cos_rep[:, st, 0:nh, :]
                        srep = sin_rep[:, st, 0:nh, :]
                        qr = pv[:, 0:2 * nh:2, :]
                        qi = pv[:, 1:2 * nh:2, :]
                        nc.vector.tensor_tensor(ta[:, 0:nh, :], qr, crep, MUL)
                        nc.vector.tensor_tensor(tb[:, 0:nh, :], qi, srep, MUL)
                        nc.vector.tensor_tensor(dstv[:, 0:2 * nh:2, :],
                                                ta[:, 0:nh, :], tb[:, 0:nh, :], SUB)
                        nc.vector.tensor_tensor(ta[:, 0:nh, :], qr, srep, MUL)
                        nc.vector.tensor_tensor(tb[:, 0:nh, :], qi, crep, MUL)
                        nc.vector.tensor_tensor(dstv[:, 1:2 * nh:2, :],
                                                ta[:, 0:nh, :], tb[:, 0:nh, :], ADD)
                    else:
                        nc.vector.tensor_copy(proj[:, st, CH * ch:CH * ch + CH],
                                              psq[:])

            # --- kv section (stores issued per chunk for an earlier trigger) ---
            proj_chunk(4)
            for dst in range(NC_CORES):
                nc.gpsimd.dma_start(
                    apkv_in[dst, :, 0:64].rearrange("(t p) m -> p t m", p=128),
                    proj[:, :, 2048 + 64 * dst:2048 + 64 * dst + 64])
            proj_chunk(5)
            for dst in range(NC_CORES):
                nc.gpsimd.dma_start(
                    apkv_in[dst, :, 64:128].rearrange("(t p) m -> p t m", p=128),
                    proj[:, :, 2560 + 64 * dst:2560 + 64 * dst + 64])
            nc.gpsimd.collective_compute(
                "AllToAll", mybir.AluOpType.bypass,
                replica_groups=groups, ins=[apkv_in.opt()], outs=[apkv_out.opt()],
            )
            # --- q pair 0 ---
            for ch in (0, 1):
                proj_chunk(ch)
                for dst in range(4 * ch, 4 * ch + 4):
                    nc.gpsimd.dma_start(
                        apq0_in[dst, :, :].rearrange("(t p) m -> p t m", p=128),
                        proj[:, :, 128 * dst:128 * dst + 128])
            nc.gpsimd.collective_compute(
                "AllToAll", mybir.AluOpType.bypass,
                replica_groups=groups, ins=[apq0_in.opt()], outs=[apq0_out.opt()],
            )
            # --- q pair 1 ---
            for ch in (2, 3):
                proj_chunk(ch)
                for dst in range(4 * (ch - 2), 4 * (ch - 2) + 4):
                    nc.gpsimd.dma_start(
                        apq1_in[dst, :, :].rearrange("(t p) m -> p t m", p=128),
                        proj[:, :, 1024 + 128 * dst:1024 + 128 * dst + 128])
            nc.gpsimd.collective_compute(
                "AllToAll", mybir.AluOpType.bypass,
                replica_groups=groups, ins=[apq1_in.opt()], outs=[apq1_out.opt()],
            )

            # ---------------- receiver: build kT / v, then qT per pair ----------
            qT_t = [[pp.tile([128, CH], BF, name=f"qT{p}_{j}")
                     for j in range(NCH)] for p in range(2)]
            kT = pp.tile([128, SEQ], BF, name="kT")
            v_sb = pp.tile([128, KT, 2 * HD], BF, name="v_sb")
            nc.gpsimd.memset(v_sb[:, :, HD:2 * HD], 1.0)

            stage_k = pp.tile([128, KT, 64], BF, name="stage_k")
            for src in range(NC_CORES):
                nc.sync.dma_start(
                    stage_k[:, 2 * src:2 * src + 2, :],
                    apkv_out[src, :, 0:64].rearrange("(t p) m -> p t m", p=128))
                nc.sync.dma_start(
                    v_sb[:, 2 * src:2 * src + 2, 0:HD],
                    apkv_out[src, :, 64:128].rearrange("(t p) m -> p t m", p=128))
            for g in range(KT):
                tk = psp.tile([64, 128], BF, tag="ps", bufs=4, name="tk")
                nc.tensor.transpose(tk[:], stage_k[:, g, :], ident[:])
                nc.vector.tensor_copy(kT[0:64, 128 * g:128 * g + 128], tk[:])
            nc.vector.tensor_copy(kT[64:128, :], kT[0:64, :])

            stage_q = pp.tile([128, 2, KT, 128], BF, name="stage_q")

            def build_qT(pair):
                apq_out = apq0_out if pair == 0 else apq1_out
                for src in range(NC_CORES):
                    nc.sync.dma_start(
                        stage_q[:, pair, 2 * src:2 * src + 2, :],
                        apq_out[src, :, :].rearrange("(t p) m -> p t m", p=128))
                    for st in range(2):
                        g = 2 * src + st
                        tq = psp.tile([128, 128], BF, tag="ps", bufs=4, name="tq")
                        tq_in = stage_q[:, pair, g, :]
                        nc.tensor.transpose(tq[:], tq_in, ident[:])
                        nc.vector.tensor_copy(
                            qT_t[pair][g // 4][:, 128 * (g % 4):128 * (g % 4) + 128],
                            tq[:])

            build_qT(0)

            # causal patterns
            patp = []
            for t in range(4):
                pat = pp.tile([128, 2, CH], BF, name=f"pat{t}")
                nc.gpsimd.memset(pat[:], 1.0)
                for half in range(2):
                    nc.gpsimd.affine_select(
                        out=pat[:, half, :], in_=pat[:, half, :],
                        compare_op=mybir.AluOpType.is_ge, fill=0.0,
                        base=-128 * t, channel_multiplier=-1, pattern=[[1, CH]],
                    )
                patp.append(pat)

            # ---------------- attention ----------------
            attnT = pp.tile([128, 2, SEQ], BF, name="attnT")

            def attention(pair, j):
                nkt = 4 * j + 4
                pso0 = psp.tile([2 * HD, CH], F32, tag="ps", bufs=4, name="pso0")
                pso1 = psp.tile([2 * HD, CH], F32, tag="ps", bufs=4, name="pso1")
                qsl = slice(CH * j, CH * j + CH)
                qTc = qT_t[pair][j]
                for kt in range(nkt):
                    ks = slice(128 * kt, 128 * kt + 128)
                    sp = psp.tile([128, 2 * CH], F32, tag="spair", bufs=2, name="sp")
                    nc.tensor.matmul(sp[:, 0:CH], kT[0:64, ks], qTc[0:64, :],
                                     start=True, stop=True)
                    nc.tensor.matmul(sp[:, CH:2 * CH], kT[64:128, ks],
                                     qTc[64:128, :], start=True, stop=True)
                    ep = wp.tile([128, 2 * CH], BF, tag="exps", bufs=4, name="ep")
                    nc.scalar.activation(ep[:], sp[:],
                                         mybir.ActivationFunctionType.Exp,
                                         scale=0.125)
                    if kt >= 4 * j:
                        pat = patp[kt - 4 * j]
                        nc.vector.tensor_tensor(ep[:], ep[:],
                                                pat[:].rearrange("p a c -> p (a c)"),
                                                MUL)
                    nc.tensor.matmul(pso0[:], v_sb[:, kt, :], ep[:, 0:CH],
                                     start=(kt == 0), stop=(kt == nkt - 1))
                    nc.tensor.matmul(pso1[:], v_sb[:, kt, :], ep[:, CH:2 * CH],
                                     start=(kt == 0), stop=(kt == nkt - 1))
                for h, pso in ((0, pso0), (1, pso1)):
                    bc = wp.tile([64, CH], F32, tag="bcast", bufs=2, name="bc")
                    nc.vector.tensor_copy(bc[:], pso[HD:2 * HD, :])
                    rc = wp.tile([64, CH], F32, tag="rcp", bufs=2, name="rc")
                    nc.vector.reciprocal_approx_fast(out=rc[:], in_=bc[:])
                    nc.vector.tensor_tensor(
                        attnT[64 * h:64 * h + 64, pair, qsl],
                        pso[0:HD, :], rc[:], MUL)

            woA = pp.tile([128, DT // 2, DIM], BF, name="woA")
            woB = pp.tile([128, DT // 2, DIM], BF, name="woB")
            for j in range(NCH):
                attention(0, j)
                if j == 1:
                    build_qT(1)   # overlaps remaining pair-0 attention
                for dst in (2 * j, 2 * j + 1):
                    nc.gpsimd.dma_start(a2a_in0[dst, :, :],
                                        attnT[:, 0, SC * dst:SC * dst + SC])
                # anchored wo prefetch (the scheduler hoists dep-free DMAs)
                nc.vector.tensor_copy(woA[0:1, 2 * j, 0:1],
                                      attnT[0:1, 0, CH * j:CH * j + 1])
                nc.sync.dma_start(
                    woA[:, 2 * j:2 * j + 2, :],
                    wo[256 * j:256 * j + 256, :].rearrange("(t p) n -> p t n",
                                                           p=128))
            nc.gpsimd.collective_compute(
                "AllToAll", mybir.AluOpType.bypass,
                replica_groups=groups, ins=[a2a_in0.opt()], outs=[a2a_out0.opt()],
            )
            a2a_sb0 = pp.tile([128, NC_CORES, SC], BF, name="a2a_sb0")
            a2a_sb1 = pp.tile([128, NC_CORES, SC], BF, name="a2a_sb1")
            for src in range(NC_CORES):
                nc.sync.dma_start(a2a_sb0[:, src, :], a2a_out0[src, :, :])
            for j in range(NCH):
                attention(1, j)
                for dst in (2 * j, 2 * j + 1):
                    nc.gpsimd.dma_start(a2a_in1[dst, :, :],
                                        attnT[:, 1, SC * dst:SC * dst + SC])
                if j < 2:
                    nc.vector.tensor_copy(woB[0:1, 4 * j, 0:1],
                                          attnT[0:1, 1, CH * j:CH * j + 1])
                    nc.sync.dma_start(
                        woB[:, 4 * j:4 * j + 4, :],
                        wo[1024 + 512 * j:1024 + 512 * j + 512, :]
                        .rearrange("(t p) n -> p t n", p=128))

            # ---------------- final A2A + output projection ----------------
            nc.gpsimd.collective_compute(
                "AllToAll", mybir.AluOpType.bypass,
                replica_groups=groups, ins=[a2a_in1.opt()], outs=[a2a_out1.opt()],
            )
            for src in range(NC_CORES):
                nc.sync.dma_start(a2a_sb1[:, src, :], a2a_out1[src, :, :])

            evens = [2 * src for src in range(NC_CORES)]
            odds = [2 * src + 1 for src in range(NC_CORES)]

            def op_mm(psf, qt, nsl, g, start, stop):
                w_ap = (woA[:, g, nsl] if g < DT // 2
                        else woB[:, g - DT // 2, nsl])
                a_ap = (a2a_sb0[:, g // 2, 128 * qt:128 * qt + 128] if g % 2 == 0
                        else a2a_sb1[:, g // 2, 128 * qt:128 * qt + 128])
                nc.tensor.matmul(psf[:], a_ap, w_ap, start=start, stop=stop)

            # pair-0 contributions as CLOSED psum groups saved to SBUF: they
            # only need the first attnT AllToAll, so the PE runs them while
            # the second is in flight (closed groups cannot be reordered
            # behind the pair-1 data). partials reuses proj's dead slot.
            partials = pp.tile([128, 2 * NCH, CH], BF, tag="proj",
                               name="partials")
            chunks = [(qt, nch) for qt in range(2) for nch in range(NCH)]
            for i8, (qt, nch) in enumerate(chunks):
                psf = psp.tile([128, CH], F32, tag="spair", bufs=2, name="psfE")
                nsl = slice(CH * nch, CH * nch + CH)
                for i, g in enumerate(evens):
                    op_mm(psf, qt, nsl, g, i == 0, i == NC_CORES - 1)
                nc.vector.tensor_copy(partials[:, i8, :], psf[:])
            for i8, (qt, nch) in enumerate(chunks):
                psf = psp.tile([128, CH], F32, tag="spair", bufs=2, name="psfO")
                nsl = slice(CH * nch, CH * nch + CH)
                for i, g in enumerate(odds):
                    op_mm(psf, qt, nsl, g, i == 0, i == NC_CORES - 1)
                osb = wp.tile([128, CH], F32, tag="osb", bufs=2, name="osb")
                nc.vector.tensor_tensor(osb[:], psf[:], partials[:, i8, :], ADD)
                nc.sync.dma_start(out[128 * qt:128 * qt + 128, nsl], osb[:])

    nc.finalize()
    return nc


def _get_nc():
    if "nc" not in _CACHE:
        _CACHE["nc"] = _build_nc()
    return _CACHE["nc"]


_PERM = np.concatenate([np.arange(0, HD, 2), np.arange(1, HD, 2)])  # de-interleave


def _shard(inputs):
    x = np.ascontiguousarray(inputs["x"][0].astype(np.float32))          # [S, D]
    wq, wk, wv = (np.asarray(inputs[k]).astype(np.float32) for k in ("wq", "wk", "wv"))
    import ml_dtypes
    wo = np.ascontiguousarray(np.asarray(inputs["wo"]).astype(ml_dtypes.bfloat16))
    cos = np.asarray(inputs["freqs_cos"]).astype(np.float32)
    sin = np.asarray(inputs["freqs_sin"]).astype(np.float32)
    # W_all columns: [q-pair0 (8x128) | q-pair1 (8x128) | k (8x64) | v (8x64)],
    # q/k head-dims de-interleaved ([32 evens | 32 odds] per head)
    wq_p = wq.reshape(DIM, 32, HD)[:, :, _PERM].reshape(DIM, 32, HD)
    wk_p = wk.reshape(DIM, 8, HD)[:, :, _PERM]
    q0 = np.concatenate([wq_p[:, 4 * c:4 * c + 2, :].reshape(DIM, 128)
                         for c in range(NC_CORES)], axis=1)
    q1 = np.concatenate([wq_p[:, 4 * c + 2:4 * c + 4, :].reshape(DIM, 128)
                         for c in range(NC_CORES)], axis=1)
    import ml_dtypes
    w_all = np.ascontiguousarray(
        np.concatenate([q0, q1, wk_p.reshape(DIM, 512), wv], axis=1)
        .astype(ml_dtypes.bfloat16))
    in_maps = []
    for c in range(NC_CORES):
        in_maps.append({
            "x_sl": np.ascontiguousarray(x[SC * c:SC * (c + 1), :]),
            "w_all": w_all,
            "wo": wo,
            "cosR": np.ascontiguousarray(cos[SC * c:SC * (c + 1), :]),
            "sinR": np.ascontiguousarray(sin[SC * c:SC * (c + 1), :]),
        })
    return in_maps


def kernel(**inputs):
    from concourse.bass_utils import run_bass_kernel_spmd

    nc = _get_nc()
    in_maps = _shard(inputs)
    res = run_bass_kernel_spmd(nc, in_maps, core_ids=list(range(NC_CORES)))
    out = np.concatenate([res.results[c]["out"] for c in range(NC_CORES)], axis=0)
    return out[None].astype(np.float32)

